# revision 3
# baseline (speedup 1.0000x reference)
"""Trainium2 Bass kernel for nn_NLL_87333864997268 (GLMM logistic NLL with
Gauss-Hermite quadrature over a random intercept).

Math
----
With y in {0,1}, f the logit, c_k = sqrt(2*sig2b)*x_k (GH nodes), the group
exponent reduces to a single 5-channel segment sum:

    T[k,q] = sum_{i in group q} [ softplus(f_i + c_k) - y_i*(f_i + c_k) ]
           ( = sum_i softplus(s_i*(f_i+c_k)), s = 1-2y, hence T >= 0 )

    loss = -sum_q log( sum_k w_k/sqrt(pi) * exp(-T[k,q]) )

Strategy
--------
Host: stable sort by group id, cut into 8 bin-aligned chunks (one per core,
no group straddles a cut => cores fully independent, no collective), and
precompute per-bin [start,end) element offsets (searchsorted).

Device (per core): elementwise softplus channels -> per-partition prefix scan
(cumsum) + triangular-matmul partition carry -> write interleaved [L+1,5]
cumulative table to DRAM -> two indirect-DMA gathers at the bin start/end
offsets -> T = diff -> stabilized log-sum-exp over the 5 quadrature nodes ->
per-core partial loss scalar. Host sums the 8 partials.
"""

import numpy as np

import concourse.bacc as bacc
import concourse.bass as bass
import concourse.mybir as mybir
import concourse.tile as tile
from concourse.bass import IndirectOffsetOnAxis
from concourse.bass_utils import run_bass_kernel_spmd
from concourse.masks import make_upper_triangular

# problem constants (hardcoded per spec)
N = 4_194_304
Q = 100_000
NCORES = 8
K = 5  # Gauss-Hermite nodes

PT = 128          # partitions
TT = 4160         # elements per partition
LP = PT * TT      # padded elements per core = 532480 (slack 8192 over N/8)
QB = 100          # bins per partition
QL = PT * QB      # padded bins per core = 12800

_XK, _WK = np.polynomial.hermite.hermgauss(K)

F32 = mybir.dt.float32
I32 = mybir.dt.int32


def build_nc():
    """Build + compile the single-core SPMD Bass program."""
    nc = bacc.Bacc("TRN2", target_bir_lowering=False, debug=False)

    ys_d = nc.dram_tensor("ys", [PT, TT], F32, kind="ExternalInput")
    fs_d = nc.dram_tensor("fs", [PT, TT], F32, kind="ExternalInput")
    ends_d = nc.dram_tensor("ends", [PT, QB], I32, kind="ExternalInput")
    starts_d = nc.dram_tensor("starts", [PT, QB], I32, kind="ExternalInput")
    cbias_d = nc.dram_tensor("cbias", [PT, K], F32, kind="ExternalInput")
    wtile_d = nc.dram_tensor("wtile", [PT, QB * K], F32, kind="ExternalInput")
    loss_d = nc.dram_tensor("loss", [1, 1], F32, kind="ExternalOutput")

    ADD = mybir.AluOpType.add
    SUB = mybir.AluOpType.subtract
    MULT = mybir.AluOpType.mult
    MIN = mybir.AluOpType.min
    ACT = mybir.ActivationFunctionType

    with tile.TileContext(nc) as tc:
        with (
            tc.tile_pool(name="big", bufs=1) as big,
            tc.tile_pool(name="tmp", bufs=2) as tmp,
            tc.tile_pool(name="small", bufs=1) as small,
            tc.tile_pool(name="psum", bufs=1, space="PSUM") as psum,
            tc.tile_pool(name="dram", bufs=1, space="DRAM") as dram,
        ):
            cumbuf = dram.tile([LP + 1, K], F32)

            yt = big.tile([PT, TT], F32, tag="yt")
            ft = big.tile([PT, TT], F32, tag="ft")
            W = big.tile([PT, TT * K], F32, tag="W")
            nc.sync.dma_start(out=yt[:], in_=ys_d[:])
            nc.sync.dma_start(out=ft[:], in_=fs_d[:])

            cb = small.tile([PT, K], F32, tag="cb")
            wt = small.tile([PT, QB * K], F32, tag="wt")
            et = small.tile([PT, QB], I32, tag="et")
            st = small.tile([PT, QB], I32, tag="st")
            nc.sync.dma_start(out=cb[:], in_=cbias_d[:])
            nc.sync.dma_start(out=wt[:], in_=wtile_d[:])
            nc.sync.dma_start(out=et[:], in_=ends_d[:])
            nc.sync.dma_start(out=st[:], in_=starts_d[:])

            tri = small.tile([PT, PT], F32, tag="tri")
            make_upper_triangular(nc, tri[:], val=1.0, diag=False)
            ones = small.tile([PT, 1], F32, tag="ones")
            nc.vector.memset(ones[:], 1.0)

            W3 = W[:].rearrange("p (t k) -> p t k", k=K)
            for k in range(K):
                sp = tmp.tile([PT, TT], F32, tag="sp")
                d1 = tmp.tile([PT, TT], F32, tag="d1")
                # sp = softplus(f + c_k) = ln(1 + exp(f + c_k))
                # (no Softplus ACT table in this build; |f+c| <= ~10 so exp is safe)
                nc.scalar.activation(
                    out=sp[:], in_=ft[:], func=ACT.Exp, bias=cb[:, k : k + 1]
                )
                nc.scalar.activation(out=sp[:], in_=sp[:], func=ACT.Ln, bias=1.0)
                # d1 = (f + c_k) * y
                nc.vector.scalar_tensor_tensor(
                    out=d1[:], in0=ft[:], scalar=cb[:, k : k + 1], in1=yt[:],
                    op0=ADD, op1=MULT,
                )
                # W[:, :, k] = cumsum_t(sp - d1)  (per-partition prefix scan)
                nc.vector.tensor_tensor_scan(
                    out=W3[:, :, k], data0=sp[:], data1=d1[:],
                    initial=0.0, op0=ADD, op1=SUB,
                )

            # cross-partition carry: offs[p,k] = sum_{p'<p} rowtotal[p',k]
            offs_p = psum.tile([PT, K], F32)
            nc.tensor.matmul(
                out=offs_p[:], lhsT=tri[:], rhs=W[:, (TT - 1) * K : TT * K],
                start=True, stop=True,
            )
            offs = small.tile([PT, K], F32, tag="offs")
            nc.vector.tensor_copy(out=offs[:], in_=offs_p[:])
            for k in range(K):
                nc.scalar.activation(
                    out=W3[:, :, k], in_=W3[:, :, k], func=ACT.Identity,
                    bias=offs[:, k : k + 1],
                )

            # cumulative table to DRAM: row 0 = zeros, row 1+j = sum of first j+1
            z = small.tile([1, K], F32, tag="z")
            nc.vector.memset(z[:], 0.0)
            nc.sync.dma_start(out=cumbuf[0:1, :], in_=z[:])
            cum_rows = cumbuf[1 : 1 + LP, :].rearrange("(p t) k -> p (t k)", p=PT)
            nc.sync.dma_start(out=cum_rows, in_=W[:])

            # gather cumulative rows at bin end/start offsets
            Ee = small.tile([PT, QB * K], F32, tag="Ee")
            Es = small.tile([PT, QB * K], F32, tag="Es")
            nc.gpsimd.indirect_dma_start(
                out=Ee[:], out_offset=None, in_=cumbuf[:],
                in_offset=IndirectOffsetOnAxis(ap=et[:], axis=0),
            )
            nc.gpsimd.indirect_dma_start(
                out=Es[:], out_offset=None, in_=cumbuf[:],
                in_offset=IndirectOffsetOnAxis(ap=st[:], axis=0),
            )

            # T[k,q] per bin; stabilized LSE over k
            S = small.tile([PT, QB * K], F32, tag="S")
            nc.vector.tensor_tensor(out=S[:], in0=Ee[:], in1=Es[:], op=SUB)
            S3 = S[:].rearrange("p (q k) -> p q k", k=K)
            tmin = small.tile([PT, QB], F32, tag="tmin")
            nc.vector.tensor_reduce(out=tmin[:], in_=S3, axis=mybir.AxisListType.X, op=MIN)
            for k in range(K):
                nc.vector.tensor_tensor(out=S3[:, :, k], in0=S3[:, :, k], in1=tmin[:], op=SUB)
            ex = small.tile([PT, QB * K], F32, tag="ex")
            nc.scalar.activation(out=ex[:], in_=S[:], func=ACT.Exp, scale=-1.0)
            wm = small.tile([PT, QB * K], F32, tag="wm")
            nc.vector.tensor_tensor(out=wm[:], in0=ex[:], in1=wt[:], op=MULT)
            ks = small.tile([PT, QB], F32, tag="ks")
            nc.vector.tensor_reduce(
                out=ks[:], in_=wm[:].rearrange("p (q k) -> p q k", k=K),
                axis=mybir.AxisListType.X, op=ADD,
            )
            lk = small.tile([PT, QB], F32, tag="lk")
            nc.scalar.activation(out=lk[:], in_=ks[:], func=ACT.Ln)
            # per-bin loss = tmin - log(ksum'); sum over bins
            dq = small.tile([PT, QB], F32, tag="dq")
            nc.vector.tensor_tensor(out=dq[:], in0=tmin[:], in1=lk[:], op=SUB)
            rs = small.tile([PT, 1], F32, tag="rs")
            nc.vector.tensor_reduce(out=rs[:], in_=dq[:], axis=mybir.AxisListType.X, op=ADD)
            tot_p = psum.tile([1, 1], F32)
            nc.tensor.matmul(out=tot_p[:], lhsT=rs[:], rhs=ones[:], start=True, stop=True)
            tot = small.tile([1, 1], F32, tag="tot")
            nc.vector.tensor_copy(out=tot[:], in_=tot_p[:])
            nc.sync.dma_start(out=loss_d[:], in_=tot[:])

    nc.compile()
    return nc


_NC_CACHE = []


def get_nc():
    if not _NC_CACHE:
        _NC_CACHE.append(build_nc())
    return _NC_CACHE[0]


def host_prep(y_true, y_pred, Z_idx, sig2b):
    """Sort by group, cut into bin-aligned chunks, build per-core inputs."""
    y = np.asarray(y_true, dtype=np.float32).reshape(-1)
    f = np.asarray(y_pred, dtype=np.float32).reshape(-1)
    idx = np.asarray(Z_idx).astype(np.int32)
    assert y.shape[0] == N and f.shape[0] == N and idx.shape[0] == N

    perm = np.argsort(idx, kind="stable")
    sb = idx[perm]
    ys = y[perm]
    fs = f[perm]

    # chunk cuts at bin boundaries, near equal N/8 splits
    cut_pos = [0]
    bstart = [0]
    for c in range(1, NCORES):
        b = int(sb[c * N // NCORES])
        pos = int(np.searchsorted(sb, b, side="left"))
        cut_pos.append(pos)
        bstart.append(b)
    cut_pos.append(N)
    bstart.append(Q)

    sig = float(np.asarray(sig2b).reshape(-1)[0])
    ck = (np.sqrt(2.0 * sig) * _XK).astype(np.float32)
    wk = (_WK / np.sqrt(np.pi)).astype(np.float32)
    cbias = np.tile(ck[None, :], (PT, 1)).astype(np.float32)
    wtile = np.tile(wk[None, :], (PT, QB)).astype(np.float32)

    in_maps = []
    for c in range(NCORES):
        p0, p1 = cut_pos[c], cut_pos[c + 1]
        b0, b1 = bstart[c], bstart[c + 1]
        lc = p1 - p0
        nbins = b1 - b0
        assert lc <= LP, f"core {c}: {lc} elements > {LP}"
        assert nbins <= QL, f"core {c}: {nbins} bins > {QL}"

        ys_c = np.zeros(LP, np.float32)
        fs_c = np.zeros(LP, np.float32)
        ys_c[:lc] = ys[p0:p1]
        fs_c[:lc] = fs[p0:p1]

        chunk = sb[p0:p1]
        ends = np.searchsorted(chunk, np.arange(b0, b0 + QL), side="right").astype(
            np.int32
        )
        starts = np.concatenate(([0], ends[:-1])).astype(np.int32)

        in_maps.append(
            {
                "ys": ys_c.reshape(PT, TT),
                "fs": fs_c.reshape(PT, TT),
                "ends": ends.reshape(PT, QB),
                "starts": starts.reshape(PT, QB),
                "cbias": cbias,
                "wtile": wtile,
            }
        )
    return in_maps


def finish(results):
    total = sum(float(results[c]["loss"][0, 0]) for c in range(NCORES))
    return np.float32(total)


def kernel(y_true, y_pred, Z_idx, sig2b):
    nc = get_nc()
    in_maps = host_prep(y_true, y_pred, Z_idx, sig2b)
    res = run_bass_kernel_spmd(nc, in_maps, list(range(NCORES)))
    return finish(res.results)


# revision 10
# speedup vs baseline: 1.7599x; 1.7599x over previous
"""Trainium2 Bass kernel for nn_NLL_87333864997268 (GLMM logistic NLL with
Gauss-Hermite quadrature over a random intercept).

Math
----
With y in {0,1}, f the logit, c_k = sqrt(2*sig2b)*x_k (GH nodes):

    T[k,q] = sum_{i in group q} [ softplus(f_i + c_k) - y_i*(f_i + c_k) ]
           = SP_k[q] - YF[q] - c_k*SY[q]        (all three are segment sums)
    loss = -sum_q log( sum_k w_k/sqrt(pi) * exp(-T[k,q]) )

Strategy
--------
Host: stable sort by group id; pad each group to fixed-width W=52 "pieces"
(ceil(size/52) pieces per group, ~+37% slots); pack pieces into 1024
partition rows (8 cores x 128 partitions), never splitting a group across
partitions. All segment sums then become dense fixed-stride reductions on
device; host supplies two {0,1} masks: m (piece j continues into j+1) and
z (piece j is the first piece of its group).

Device (per core): 3 dense reduces for y and y*f, then per quadrature node
softplus (exp+ln, no Softplus table in this build) and a dense piece-sum
reduce; combine pieces via the m-mask (twice, supporting groups up to 3
pieces); stabilized log-sum-exp over the 5 nodes; z-masked total -> one
scalar per core. Host sums the 8 partials. Pad slots use f=-1e4, y=0 so
softplus(f+c)=0 exactly and they contribute nothing.
"""

import numpy as np

import concourse.bacc as bacc
import concourse.bass as bass
import concourse.mybir as mybir
import concourse.tile as tile
from concourse.bass_utils import run_bass_kernel_spmd

# problem constants (hardcoded per spec)
N = 4_194_304
Q = 100_000
NCORES = 8
K = 5            # Gauss-Hermite nodes
PT = 128         # partitions per core
NPART = NCORES * PT

WB = 52          # piece width (slots per piece)
NP = 108         # pieces per partition (static capacity)
FT = NP * WB     # slots per partition = 5616
FPAD = -10000.0  # pad logit: softplus(fpad+c)=0, y=0

_XK, _WK = np.polynomial.hermite.hermgauss(K)

F32 = mybir.dt.float32


def build_nc(dbg=False):
    """Build + compile the single-core SPMD Bass program."""
    nc = bacc.Bacc("TRN2", target_bir_lowering=False, debug=False)

    ys_d = nc.dram_tensor("ys", [PT, FT], F32, kind="ExternalInput")
    fs_d = nc.dram_tensor("fs", [PT, FT], F32, kind="ExternalInput")
    m5_d = nc.dram_tensor("m5", [PT, NP * K], F32, kind="ExternalInput")
    z_d = nc.dram_tensor("z", [PT, NP], F32, kind="ExternalInput")
    cbias_d = nc.dram_tensor("cbias", [PT, K], F32, kind="ExternalInput")
    wtile_d = nc.dram_tensor("wtile", [PT, NP * K], F32, kind="ExternalInput")
    loss_d = nc.dram_tensor("loss", [1, 1], F32, kind="ExternalOutput")
    dbg_t = {}
    if dbg:
        dbg_t["dbg_T"] = nc.dram_tensor("dbg_T", [PT, NP * K], F32, kind="ExternalOutput")
        dbg_t["dbg_P"] = nc.dram_tensor("dbg_P", [PT, NP * K], F32, kind="ExternalOutput")
        dbg_t["dbg_dq"] = nc.dram_tensor("dbg_dq", [PT, NP], F32, kind="ExternalOutput")

    ADD = mybir.AluOpType.add
    SUB = mybir.AluOpType.subtract
    MULT = mybir.AluOpType.mult
    MIN = mybir.AluOpType.min
    ACT = mybir.ActivationFunctionType
    AX = mybir.AxisListType.X

    with tile.TileContext(nc) as tc:
        with (
            tc.tile_pool(name="big", bufs=1) as big,
            tc.tile_pool(name="tmp", bufs=2) as tmp,
            tc.tile_pool(name="small", bufs=1) as small,
            tc.tile_pool(name="psum", bufs=1, space="PSUM") as psum,
        ):
            yt = big.tile([PT, FT], F32, tag="yt")
            ft = big.tile([PT, FT], F32, tag="ft")
            nc.sync.dma_start(out=yt[:], in_=ys_d[:])
            nc.sync.dma_start(out=ft[:], in_=fs_d[:])

            cb = small.tile([PT, K], F32, tag="cb")
            m5 = small.tile([PT, NP * K], F32, tag="m5")
            zt = small.tile([PT, NP], F32, tag="zt")
            wt = small.tile([PT, NP * K], F32, tag="wt")
            nc.sync.dma_start(out=cb[:], in_=cbias_d[:])
            nc.sync.dma_start(out=m5[:], in_=m5_d[:])
            nc.sync.dma_start(out=zt[:], in_=z_d[:])
            nc.sync.dma_start(out=wt[:], in_=wtile_d[:])

            ones = small.tile([PT, 1], F32, tag="ones")
            nc.vector.memset(ones[:], 1.0)

            # piece sums of y and y*f
            yf = big.tile([PT, FT], F32, tag="yf")
            nc.vector.tensor_tensor(out=yf[:], in0=yt[:], in1=ft[:], op=MULT)
            sy = small.tile([PT, NP], F32, tag="sy")
            syf = small.tile([PT, NP], F32, tag="syf")
            nc.vector.tensor_reduce(
                out=sy[:], in_=yt[:].rearrange("p (n w) -> p n w", w=WB), axis=AX, op=ADD
            )
            nc.vector.tensor_reduce(
                out=syf[:], in_=yf[:].rearrange("p (n w) -> p n w", w=WB), axis=AX, op=ADD
            )

            # per-node piece sums of softplus(f + c_k); T pieces
            T = small.tile([PT, NP * K], F32, tag="T")
            T3 = T[:].rearrange("p (n k) -> p n k", k=K)
            for k in range(K):
                sp = tmp.tile([PT, FT], F32, tag="sp")
                nc.scalar.activation(out=sp[:], in_=ft[:], func=ACT.Exp, bias=cb[:, k : k + 1])
                nc.scalar.activation(out=sp[:], in_=sp[:], func=ACT.Ln, bias=1.0)
                spn = tmp.tile([PT, NP], F32, tag="spn")
                nc.vector.tensor_reduce(
                    out=spn[:], in_=sp[:].rearrange("p (n w) -> p n w", w=WB), axis=AX, op=ADD
                )
                # T_k = spn - syf - c_k*sy   (per piece)
                t1 = tmp.tile([PT, NP], F32, tag="t1")
                nc.vector.scalar_tensor_tensor(
                    out=t1[:], in0=sy[:], scalar=cb[:, k : k + 1], in1=spn[:],
                    op0=MULT, op1=SUB,
                )
                # t1 = c_k*sy - spn  -> T_k = -(t1 + syf) ... avoid negation:
                # instead compute T_k = (sy*(-c_k) + spn) - syf with negated scalar.
                # simpler: T_k = spn - syf - c_k*sy done in two ops:
                nc.vector.tensor_tensor(out=T3[:, :, k], in0=t1[:], in1=syf[:], op=ADD)
            # NOTE: T currently holds  c_k*sy - spn + syf  = -(T_k). We work with
            # negT below: ksum = sum_k w_k * exp(negT - negTmax), loss_q = -negTmax - ln ksum.
            negT = T

            # piece combine: PC_j = negT_j + m_j*(negT_{j+1} + m_{j+1}*negT_{j+2})
            # (supports groups spanning up to 3 pieces; host asserts that)
            negT_s = small.tile([PT, (NP + 1) * K], F32, tag="negT_s")
            nc.vector.memset(negT_s[:, NP * K :], 0.0)
            nc.vector.tensor_copy(out=negT_s[:, : NP * K], in_=negT[:])
            pc1 = small.tile([PT, NP * K], F32, tag="pc1")
            t2 = tmp.tile([PT, NP * K], F32, tag="t2")
            nc.vector.tensor_tensor(out=t2[:], in0=negT_s[:, K : (NP + 1) * K], in1=m5[:], op=MULT)
            nc.vector.tensor_tensor(out=pc1[:], in0=negT[:], in1=t2[:], op=ADD)
            pc1_s = small.tile([PT, (NP + 1) * K], F32, tag="pc1_s")
            nc.vector.memset(pc1_s[:, NP * K :], 0.0)
            nc.vector.tensor_copy(out=pc1_s[:, : NP * K], in_=pc1[:])
            pc = small.tile([PT, NP * K], F32, tag="pc")
            t3 = tmp.tile([PT, NP * K], F32, tag="t3")
            nc.vector.tensor_tensor(out=t3[:], in0=pc1_s[:, K : (NP + 1) * K], in1=m5[:], op=MULT)
            nc.vector.tensor_tensor(out=pc[:], in0=negT[:], in1=t3[:], op=ADD)

            # stabilized LSE over k on combined pieces
            nmax = small.tile([PT, NP], F32, tag="nmax")
            nc.vector.tensor_reduce(
                out=nmax[:], in_=pc[:].rearrange("p (n k) -> p n k", k=K),
                axis=AX, op=mybir.AluOpType.max,
            )
            pc3 = pc[:].rearrange("p (n k) -> p n k", k=K)
            for k in range(K):
                nc.vector.tensor_tensor(out=pc3[:, :, k], in0=pc3[:, :, k], in1=nmax[:], op=SUB)
            ex = small.tile([PT, NP * K], F32, tag="ex")
            nc.scalar.activation(out=ex[:], in_=pc[:], func=ACT.Exp)
            wm = small.tile([PT, NP * K], F32, tag="wm")
            nc.vector.tensor_tensor(out=wm[:], in0=ex[:], in1=wt[:], op=MULT)
            ks = small.tile([PT, NP], F32, tag="ks")
            nc.vector.tensor_reduce(
                out=ks[:], in_=wm[:].rearrange("p (n k) -> p n k", k=K), axis=AX, op=ADD
            )
            lk = small.tile([PT, NP], F32, tag="lk")
            nc.scalar.activation(out=lk[:], in_=ks[:], func=ACT.Ln)
            # loss_q = -log ksum_full = -(nmax + lk)  => accumulate (nmax+lk), negate at end
            dq = small.tile([PT, NP], F32, tag="dq")
            nc.vector.tensor_tensor(out=dq[:], in0=nmax[:], in1=lk[:], op=ADD)
            dqz = small.tile([PT, NP], F32, tag="dqz")
            nc.vector.tensor_tensor(out=dqz[:], in0=dq[:], in1=zt[:], op=MULT)
            if dbg:
                nc.sync.dma_start(out=dbg_t["dbg_T"][:], in_=negT[:])
                nc.sync.dma_start(out=dbg_t["dbg_P"][:], in_=pc[:])
                nc.sync.dma_start(out=dbg_t["dbg_dq"][:], in_=dqz[:])
            rs = small.tile([PT, 1], F32, tag="rs")
            nc.vector.tensor_reduce(out=rs[:], in_=dqz[:], axis=AX, op=ADD)
            negones = small.tile([PT, 1], F32, tag="negones")
            nc.vector.memset(negones[:], -1.0)
            tot_p = psum.tile([1, 1], F32)
            nc.tensor.matmul(out=tot_p[:], lhsT=rs[:], rhs=negones[:], start=True, stop=True)
            tot = small.tile([1, 1], F32, tag="tot")
            nc.vector.tensor_copy(out=tot[:], in_=tot_p[:])
            nc.sync.dma_start(out=loss_d[:], in_=tot[:])

    nc.compile()
    return nc


_NC_CACHE = {}


def get_nc(dbg=False):
    if dbg not in _NC_CACHE:
        _NC_CACHE[dbg] = build_nc(dbg)
    return _NC_CACHE[dbg]


def host_prep(y_true, y_pred, Z_idx, sig2b):
    """Sort by group; pack groups into fixed-width pieces across 1024 partitions."""
    y = np.asarray(y_true, dtype=np.float32).reshape(-1)
    f = np.asarray(y_pred, dtype=np.float32).reshape(-1)
    idx = np.asarray(Z_idx).astype(np.int32)
    n = y.shape[0]
    assert n == N

    perm = np.argsort(idx, kind="stable")
    sb = idx[perm]
    ys = y[perm]
    fs = f[perm]

    s = np.bincount(sb, minlength=Q).astype(np.int64)          # group sizes
    bin_start = np.concatenate(([0], np.cumsum(s)[:-1]))
    pcs = (s + WB - 1) // WB                                   # pieces per group
    piece_off = np.concatenate(([0], np.cumsum(pcs)[:-1]))
    total_pieces = int(pcs.sum())
    npt = -(-total_pieces // NPART)                            # target pieces/partition
    assert npt + int(pcs.max()) - 1 <= NP, (npt, int(pcs.max()))
    assert int(pcs.max()) <= 3, int(pcs.max())                 # combine depth

    nz = s > 0
    pid = np.zeros(Q, np.int64)
    pid[nz] = piece_off[nz] // npt                             # partition of each group
    assert pid.max() < NPART

    # local piece base per group: piece_off - first piece_off in its partition
    first_bin = np.searchsorted(pid[nz], np.arange(NPART), side="left")
    po_nz = piece_off[nz]
    part_first = np.zeros(NPART, np.int64)
    valid = first_bin < po_nz.shape[0]
    part_first[valid] = po_nz[np.minimum(first_bin, po_nz.shape[0] - 1)][valid]
    lpi = np.zeros(Q, np.int64)
    lpi[nz] = piece_off[nz] - part_first[pid[nz]]
    assert (lpi[nz] + pcs[nz]).max() <= NP

    # per-element slot
    b = sb.astype(np.int64)
    r = np.arange(n, dtype=np.int64) - bin_start[b]            # rank within group
    slot = pid[b] * FT + (lpi[b] + r // WB) * WB + (r % WB)

    Y = np.zeros(NPART * FT, np.float32)
    F = np.full(NPART * FT, FPAD, np.float32)
    Y[slot] = ys
    F[slot] = fs

    # masks
    mflat = np.zeros(NPART * NP, np.float32)
    for extra in (1, 2):
        sel = pcs > extra
        mflat[(pid[sel] * NP + lpi[sel] + (extra - 1)).astype(np.int64)] = 1.0
    zflat = np.zeros(NPART * NP, np.float32)
    zflat[(pid[nz] * NP + lpi[nz]).astype(np.int64)] = 1.0

    sig = float(np.asarray(sig2b).reshape(-1)[0])
    ck = (np.sqrt(2.0 * sig) * _XK).astype(np.float32)
    wk = (_WK / np.sqrt(np.pi)).astype(np.float32)
    cbias = np.tile(ck[None, :], (PT, 1)).astype(np.float32)
    wtile = np.tile(wk[None, :], (PT, NP)).astype(np.float32)

    Y = Y.reshape(NPART, FT)
    F = F.reshape(NPART, FT)
    m5 = np.repeat(mflat.reshape(NPART, NP), K, axis=1)        # [NPART, NP*K]
    z2 = zflat.reshape(NPART, NP)

    in_maps = []
    for c in range(NCORES):
        sl = slice(c * PT, (c + 1) * PT)
        in_maps.append(
            {
                "ys": Y[sl],
                "fs": F[sl],
                "m5": m5[sl],
                "z": z2[sl],
                "cbias": cbias,
                "wtile": wtile,
            }
        )
    return in_maps


def finish(results):
    total = sum(float(results[c]["loss"][0, 0]) for c in range(NCORES))
    return np.float32(total)


def kernel(y_true, y_pred, Z_idx, sig2b):
    nc = get_nc()
    in_maps = host_prep(y_true, y_pred, Z_idx, sig2b)
    res = run_bass_kernel_spmd(nc, in_maps, list(range(NCORES)))
    return finish(res.results)


# revision 13
# speedup vs baseline: 2.3357x; 1.3272x over previous
"""Trainium2 Bass kernel for nn_NLL_87333864997268 (GLMM logistic NLL with
Gauss-Hermite quadrature over a random intercept).

Math
----
With y in {0,1}, f the logit, c_k = sqrt(2*sig2b)*x_k (GH nodes):

    T[k,q] = sum_{i in group q} [ softplus(f_i + c_k) - y_i*(f_i + c_k) ]
           = SP_k[q] - YF[q] - c_k*SY[q]        (all three are segment sums)
    loss = -sum_q log( sum_k w_k/sqrt(pi) * exp(-T[k,q]) )

Strategy
--------
Host: stable sort by group id; pad each group to fixed-width W=52 "pieces"
(ceil(size/52) pieces per group, ~+37% slots); pack pieces into 1024
partition rows (8 cores x 128 partitions), never splitting a group across
partitions. All segment sums then become dense fixed-stride reductions on
device; host supplies two {0,1} masks: m (piece j continues into j+1) and
z (piece j is the first piece of its group).

Device (per core): 3 dense reduces for y and y*f, then per quadrature node
softplus (exp+ln, no Softplus table in this build) and a dense piece-sum
reduce; combine pieces via the m-mask (twice, supporting groups up to 3
pieces); stabilized log-sum-exp over the 5 nodes; z-masked total -> one
scalar per core. Host sums the 8 partials. Pad slots use f=-1e4, y=0 so
softplus(f+c)=0 exactly and they contribute nothing.
"""

import numpy as np

import concourse.bacc as bacc
import concourse.bass as bass
import concourse.mybir as mybir
import concourse.tile as tile
from concourse.bass_utils import run_bass_kernel_spmd

# problem constants (hardcoded per spec)
N = 4_194_304
Q = 100_000
NCORES = 8
K = 5            # Gauss-Hermite nodes
PT = 128         # partitions per core
NPART = NCORES * PT

WB = 52          # piece width (slots per piece)
NP = 108         # pieces per partition (static capacity)
FT = NP * WB     # slots per partition = 5616
FPAD = -10000.0  # pad logit: softplus(fpad+c)=0, y=0

_XK, _WK = np.polynomial.hermite.hermgauss(K)

F32 = mybir.dt.float32
BF16 = mybir.dt.bfloat16


def build_nc(dbg=False):
    """Build + compile the single-core SPMD Bass program."""
    nc = bacc.Bacc("TRN2", target_bir_lowering=False, debug=False)

    ys_d = nc.dram_tensor("ys", [PT, FT], BF16, kind="ExternalInput")
    fs_d = nc.dram_tensor("fs", [PT, FT], BF16, kind="ExternalInput")
    m5_d = nc.dram_tensor("m5", [PT, NP * K], F32, kind="ExternalInput")
    z_d = nc.dram_tensor("z", [PT, NP], F32, kind="ExternalInput")
    cbias_d = nc.dram_tensor("cbias", [PT, K], F32, kind="ExternalInput")
    escale_d = nc.dram_tensor("escale", [PT, K], F32, kind="ExternalInput")
    wtile_d = nc.dram_tensor("wtile", [PT, NP * K], F32, kind="ExternalInput")
    loss_d = nc.dram_tensor("loss", [1, 1], F32, kind="ExternalOutput")
    dbg_t = {}
    if dbg:
        dbg_t["dbg_T"] = nc.dram_tensor("dbg_T", [PT, NP * K], F32, kind="ExternalOutput")
        dbg_t["dbg_P"] = nc.dram_tensor("dbg_P", [PT, NP * K], F32, kind="ExternalOutput")
        dbg_t["dbg_dq"] = nc.dram_tensor("dbg_dq", [PT, NP], F32, kind="ExternalOutput")

    ADD = mybir.AluOpType.add
    SUB = mybir.AluOpType.subtract
    MULT = mybir.AluOpType.mult
    MIN = mybir.AluOpType.min
    ACT = mybir.ActivationFunctionType
    AX = mybir.AxisListType.X

    with tile.TileContext(nc) as tc:
        with (
            tc.tile_pool(name="big", bufs=1) as big,
            tc.tile_pool(name="tmp", bufs=2) as tmp,
            tc.tile_pool(name="small", bufs=1) as small,
            tc.tile_pool(name="psum", bufs=1, space="PSUM") as psum,
        ):
            yt = big.tile([PT, FT], BF16, tag="yt")
            ft = big.tile([PT, FT], BF16, tag="ft")
            nc.sync.dma_start(out=yt[:], in_=ys_d[:])
            nc.sync.dma_start(out=ft[:], in_=fs_d[:])

            cb = small.tile([PT, K], F32, tag="cb")
            es = small.tile([PT, K], F32, tag="es")
            m5 = small.tile([PT, NP * K], F32, tag="m5")
            zt = small.tile([PT, NP], F32, tag="zt")
            wt = small.tile([PT, NP * K], F32, tag="wt")
            nc.sync.dma_start(out=cb[:], in_=cbias_d[:])
            nc.sync.dma_start(out=es[:], in_=escale_d[:])
            nc.sync.dma_start(out=m5[:], in_=m5_d[:])
            nc.sync.dma_start(out=zt[:], in_=z_d[:])
            nc.sync.dma_start(out=wt[:], in_=wtile_d[:])

            # piece sums of y and y*f (y*f on gpsimd; DVE is the critical engine)
            yf = big.tile([PT, FT], BF16, tag="yf")
            nc.gpsimd.tensor_tensor(out=yf[:], in0=yt[:], in1=ft[:], op=MULT)
            sy = small.tile([PT, NP], F32, tag="sy")
            syf = small.tile([PT, NP], F32, tag="syf")
            nc.vector.tensor_reduce(
                out=sy[:], in_=yt[:].rearrange("p (n w) -> p n w", w=WB), axis=AX, op=ADD
            )
            nc.vector.tensor_reduce(
                out=syf[:], in_=yf[:].rearrange("p (n w) -> p n w", w=WB), axis=AX, op=ADD
            )

            # e^f once; softplus(f+c_k) = ln(e^{c_k} * e^f + 1)
            ef = big.tile([PT, FT], BF16, tag="ef")
            nc.scalar.activation(out=ef[:], in_=ft[:], func=ACT.Exp)

            # per-node piece sums of softplus(f + c_k); T pieces
            T = small.tile([PT, NP * K], F32, tag="T")
            T3 = T[:].rearrange("p (n k) -> p n k", k=K)
            for k in range(K):
                sp = tmp.tile([PT, FT], BF16, tag="sp")
                nc.scalar.activation(
                    out=sp[:], in_=ef[:], func=ACT.Ln, bias=1.0, scale=es[:, k : k + 1]
                )
                spn = tmp.tile([PT, NP], F32, tag="spn")
                nc.vector.tensor_reduce(
                    out=spn[:], in_=sp[:].rearrange("p (n w) -> p n w", w=WB), axis=AX, op=ADD
                )
                # T_k = spn - syf - c_k*sy   (per piece)
                t1 = tmp.tile([PT, NP], F32, tag="t1")
                nc.vector.scalar_tensor_tensor(
                    out=t1[:], in0=sy[:], scalar=cb[:, k : k + 1], in1=spn[:],
                    op0=MULT, op1=SUB,
                )
                # t1 = c_k*sy - spn  -> T_k = -(t1 + syf) ... avoid negation:
                # instead compute T_k = (sy*(-c_k) + spn) - syf with negated scalar.
                # simpler: T_k = spn - syf - c_k*sy done in two ops:
                nc.vector.tensor_tensor(out=T3[:, :, k], in0=t1[:], in1=syf[:], op=ADD)
            # NOTE: T currently holds  c_k*sy - spn + syf  = -(T_k). We work with
            # negT below: ksum = sum_k w_k * exp(negT - negTmax), loss_q = -negTmax - ln ksum.
            negT = T

            # piece combine: PC_j = negT_j + m_j*(negT_{j+1} + m_{j+1}*negT_{j+2})
            # (supports groups spanning up to 3 pieces; host asserts that)
            negT_s = small.tile([PT, (NP + 1) * K], F32, tag="negT_s")
            nc.vector.memset(negT_s[:, NP * K :], 0.0)
            nc.vector.tensor_copy(out=negT_s[:, : NP * K], in_=negT[:])
            pc1 = small.tile([PT, NP * K], F32, tag="pc1")
            t2 = tmp.tile([PT, NP * K], F32, tag="t2")
            nc.vector.tensor_tensor(out=t2[:], in0=negT_s[:, K : (NP + 1) * K], in1=m5[:], op=MULT)
            nc.vector.tensor_tensor(out=pc1[:], in0=negT[:], in1=t2[:], op=ADD)
            pc1_s = small.tile([PT, (NP + 1) * K], F32, tag="pc1_s")
            nc.vector.memset(pc1_s[:, NP * K :], 0.0)
            nc.vector.tensor_copy(out=pc1_s[:, : NP * K], in_=pc1[:])
            pc = small.tile([PT, NP * K], F32, tag="pc")
            t3 = tmp.tile([PT, NP * K], F32, tag="t3")
            nc.vector.tensor_tensor(out=t3[:], in0=pc1_s[:, K : (NP + 1) * K], in1=m5[:], op=MULT)
            nc.vector.tensor_tensor(out=pc[:], in0=negT[:], in1=t3[:], op=ADD)

            # stabilized LSE over k on combined pieces
            nmax = small.tile([PT, NP], F32, tag="nmax")
            nc.vector.tensor_reduce(
                out=nmax[:], in_=pc[:].rearrange("p (n k) -> p n k", k=K),
                axis=AX, op=mybir.AluOpType.max,
            )
            pc3 = pc[:].rearrange("p (n k) -> p n k", k=K)
            for k in range(K):
                nc.vector.tensor_tensor(out=pc3[:, :, k], in0=pc3[:, :, k], in1=nmax[:], op=SUB)
            ex = small.tile([PT, NP * K], F32, tag="ex")
            nc.scalar.activation(out=ex[:], in_=pc[:], func=ACT.Exp)
            wm = small.tile([PT, NP * K], F32, tag="wm")
            nc.vector.tensor_tensor(out=wm[:], in0=ex[:], in1=wt[:], op=MULT)
            ks = small.tile([PT, NP], F32, tag="ks")
            nc.vector.tensor_reduce(
                out=ks[:], in_=wm[:].rearrange("p (n k) -> p n k", k=K), axis=AX, op=ADD
            )
            lk = small.tile([PT, NP], F32, tag="lk")
            nc.scalar.activation(out=lk[:], in_=ks[:], func=ACT.Ln)
            # loss_q = -log ksum_full = -(nmax + lk)  => accumulate (nmax+lk), negate at end
            dq = small.tile([PT, NP], F32, tag="dq")
            nc.vector.tensor_tensor(out=dq[:], in0=nmax[:], in1=lk[:], op=ADD)
            dqz = small.tile([PT, NP], F32, tag="dqz")
            nc.vector.tensor_tensor(out=dqz[:], in0=dq[:], in1=zt[:], op=MULT)
            if dbg:
                nc.sync.dma_start(out=dbg_t["dbg_T"][:], in_=negT[:])
                nc.sync.dma_start(out=dbg_t["dbg_P"][:], in_=pc[:])
                nc.sync.dma_start(out=dbg_t["dbg_dq"][:], in_=dqz[:])
            rs = small.tile([PT, 1], F32, tag="rs")
            nc.vector.tensor_reduce(out=rs[:], in_=dqz[:], axis=AX, op=ADD)
            negones = small.tile([PT, 1], F32, tag="negones")
            nc.vector.memset(negones[:], -1.0)
            tot_p = psum.tile([1, 1], F32)
            nc.tensor.matmul(out=tot_p[:], lhsT=rs[:], rhs=negones[:], start=True, stop=True)
            tot = small.tile([1, 1], F32, tag="tot")
            nc.vector.tensor_copy(out=tot[:], in_=tot_p[:])
            nc.sync.dma_start(out=loss_d[:], in_=tot[:])

    nc.compile()
    return nc


_NC_CACHE = {}


def get_nc(dbg=False):
    if dbg not in _NC_CACHE:
        _NC_CACHE[dbg] = build_nc(dbg)
    return _NC_CACHE[dbg]


def host_prep(y_true, y_pred, Z_idx, sig2b):
    """Sort by group; pack groups into fixed-width pieces across 1024 partitions."""
    y = np.asarray(y_true, dtype=np.float32).reshape(-1)
    f = np.asarray(y_pred, dtype=np.float32).reshape(-1)
    idx = np.asarray(Z_idx).astype(np.int32)
    n = y.shape[0]
    assert n == N

    perm = np.argsort(idx, kind="stable")
    sb = idx[perm]
    ys = y[perm]
    fs = f[perm]

    s = np.bincount(sb, minlength=Q).astype(np.int64)          # group sizes
    bin_start = np.concatenate(([0], np.cumsum(s)[:-1]))
    pcs = (s + WB - 1) // WB                                   # pieces per group
    piece_off = np.concatenate(([0], np.cumsum(pcs)[:-1]))
    total_pieces = int(pcs.sum())
    npt = -(-total_pieces // NPART)                            # target pieces/partition
    assert npt + int(pcs.max()) - 1 <= NP, (npt, int(pcs.max()))
    assert int(pcs.max()) <= 3, int(pcs.max())                 # combine depth

    nz = s > 0
    pid = np.zeros(Q, np.int64)
    pid[nz] = piece_off[nz] // npt                             # partition of each group
    assert pid.max() < NPART

    # local piece base per group: piece_off - first piece_off in its partition
    first_bin = np.searchsorted(pid[nz], np.arange(NPART), side="left")
    po_nz = piece_off[nz]
    part_first = np.zeros(NPART, np.int64)
    valid = first_bin < po_nz.shape[0]
    part_first[valid] = po_nz[np.minimum(first_bin, po_nz.shape[0] - 1)][valid]
    lpi = np.zeros(Q, np.int64)
    lpi[nz] = piece_off[nz] - part_first[pid[nz]]
    assert (lpi[nz] + pcs[nz]).max() <= NP

    # per-element slot
    b = sb.astype(np.int64)
    r = np.arange(n, dtype=np.int64) - bin_start[b]            # rank within group
    slot = pid[b] * FT + (lpi[b] + r // WB) * WB + (r % WB)

    Y = np.zeros(NPART * FT, np.float32)
    F = np.full(NPART * FT, FPAD, np.float32)
    Y[slot] = ys
    F[slot] = fs

    # masks
    mflat = np.zeros(NPART * NP, np.float32)
    for extra in (1, 2):
        sel = pcs > extra
        mflat[(pid[sel] * NP + lpi[sel] + (extra - 1)).astype(np.int64)] = 1.0
    zflat = np.zeros(NPART * NP, np.float32)
    zflat[(pid[nz] * NP + lpi[nz]).astype(np.int64)] = 1.0

    sig = float(np.asarray(sig2b).reshape(-1)[0])
    ck = (np.sqrt(2.0 * sig) * _XK).astype(np.float32)
    wk = (_WK / np.sqrt(np.pi)).astype(np.float32)
    cbias = np.tile(ck[None, :], (PT, 1)).astype(np.float32)
    escale = np.tile(np.exp(ck.astype(np.float64))[None, :], (PT, 1)).astype(np.float32)
    wtile = np.tile(wk[None, :], (PT, NP)).astype(np.float32)

    bf16 = mybir.dt.np(BF16)
    Y = Y.reshape(NPART, FT).astype(bf16)
    F = F.reshape(NPART, FT).astype(bf16)
    m5 = np.repeat(mflat.reshape(NPART, NP), K, axis=1)        # [NPART, NP*K]
    z2 = zflat.reshape(NPART, NP)

    in_maps = []
    for c in range(NCORES):
        sl = slice(c * PT, (c + 1) * PT)
        in_maps.append(
            {
                "ys": Y[sl],
                "fs": F[sl],
                "m5": m5[sl],
                "z": z2[sl],
                "cbias": cbias,
                "escale": escale,
                "wtile": wtile,
            }
        )
    return in_maps


def finish(results):
    total = sum(float(results[c]["loss"][0, 0]) for c in range(NCORES))
    return np.float32(total)


def kernel(y_true, y_pred, Z_idx, sig2b):
    nc = get_nc()
    in_maps = host_prep(y_true, y_pred, Z_idx, sig2b)
    res = run_bass_kernel_spmd(nc, in_maps, list(range(NCORES)))
    return finish(res.results)


# revision 15
# speedup vs baseline: 2.3387x; 1.0013x over previous
"""Trainium2 Bass kernel for nn_NLL_87333864997268 (GLMM logistic NLL with
Gauss-Hermite quadrature over a random intercept).

Math
----
With y in {0,1}, f the logit, c_k = sqrt(2*sig2b)*x_k (GH nodes):

    T[k,q] = sum_{i in group q} [ softplus(f_i + c_k) - y_i*(f_i + c_k) ]
           = SP_k[q] - YF[q] - c_k*SY[q]        (all three are segment sums)
    loss = -sum_q log( sum_k w_k/sqrt(pi) * exp(-T[k,q]) )

Strategy
--------
Host: stable sort by group id; pad each group to fixed-width W=52 "pieces"
(ceil(size/52) pieces per group, ~+37% slots); pack pieces into 1024
partition rows (8 cores x 128 partitions), never splitting a group across
partitions. All segment sums then become dense fixed-stride reductions on
device; host supplies two {0,1} masks: m (piece j continues into j+1) and
z (piece j is the first piece of its group).

Device (per core): 3 dense reduces for y and y*f, then per quadrature node
softplus (exp+ln, no Softplus table in this build) and a dense piece-sum
reduce; combine pieces via the m-mask (twice, supporting groups up to 3
pieces); stabilized log-sum-exp over the 5 nodes; z-masked total -> one
scalar per core. Host sums the 8 partials. Pad slots use f=-1e4, y=0 so
softplus(f+c)=0 exactly and they contribute nothing.
"""

import numpy as np

import concourse.bacc as bacc
import concourse.bass as bass
import concourse.mybir as mybir
import concourse.tile as tile
from concourse.bass_utils import run_bass_kernel_spmd

# problem constants (hardcoded per spec)
N = 4_194_304
Q = 100_000
NCORES = 8
K = 5            # Gauss-Hermite nodes
PT = 128         # partitions per core
NPART = NCORES * PT

WB = 52          # piece width (slots per piece)
NP = 108         # pieces per partition (static capacity)
FT = NP * WB     # slots per partition = 5616
FPAD = -10000.0  # pad logit: softplus(fpad+c)=0, y=0

_XK, _WK = np.polynomial.hermite.hermgauss(K)

F32 = mybir.dt.float32
BF16 = mybir.dt.bfloat16


def build_nc(dbg=False):
    """Build + compile the single-core SPMD Bass program."""
    nc = bacc.Bacc("TRN2", target_bir_lowering=False, debug=False)

    ys_d = nc.dram_tensor("ys", [PT, FT], BF16, kind="ExternalInput")
    fs_d = nc.dram_tensor("fs", [PT, FT], BF16, kind="ExternalInput")
    m5_d = nc.dram_tensor("m5", [PT, NP * K], F32, kind="ExternalInput")
    z_d = nc.dram_tensor("z", [PT, NP], F32, kind="ExternalInput")
    cbias_d = nc.dram_tensor("cbias", [PT, K], F32, kind="ExternalInput")
    escale_d = nc.dram_tensor("escale", [PT, K], F32, kind="ExternalInput")
    wtile_d = nc.dram_tensor("wtile", [PT, NP * K], F32, kind="ExternalInput")
    loss_d = nc.dram_tensor("loss", [1, 1], F32, kind="ExternalOutput")
    dbg_t = {}
    if dbg:
        dbg_t["dbg_T"] = nc.dram_tensor("dbg_T", [PT, NP * K], F32, kind="ExternalOutput")
        dbg_t["dbg_P"] = nc.dram_tensor("dbg_P", [PT, NP * K], F32, kind="ExternalOutput")
        dbg_t["dbg_dq"] = nc.dram_tensor("dbg_dq", [PT, NP], F32, kind="ExternalOutput")

    ADD = mybir.AluOpType.add
    SUB = mybir.AluOpType.subtract
    MULT = mybir.AluOpType.mult
    MIN = mybir.AluOpType.min
    ACT = mybir.ActivationFunctionType
    AX = mybir.AxisListType.X

    with tile.TileContext(nc) as tc:
        with (
            tc.tile_pool(name="big", bufs=1) as big,
            tc.tile_pool(name="tmp", bufs=2) as tmp,
            tc.tile_pool(name="small", bufs=1) as small,
            tc.tile_pool(name="psum", bufs=1, space="PSUM") as psum,
        ):
            yt = big.tile([PT, FT], BF16, tag="yt")
            ft = big.tile([PT, FT], BF16, tag="ft")
            nc.sync.dma_start(out=yt[:], in_=ys_d[:])
            nc.sync.dma_start(out=ft[:], in_=fs_d[:])

            cb = small.tile([PT, K], F32, tag="cb")
            es = small.tile([PT, K], F32, tag="es")
            m5 = small.tile([PT, NP * K], F32, tag="m5")
            zt = small.tile([PT, NP], F32, tag="zt")
            wt = small.tile([PT, NP * K], F32, tag="wt")
            nc.sync.dma_start(out=cb[:], in_=cbias_d[:])
            nc.sync.dma_start(out=es[:], in_=escale_d[:])
            nc.sync.dma_start(out=m5[:], in_=m5_d[:])
            nc.sync.dma_start(out=zt[:], in_=z_d[:])
            nc.sync.dma_start(out=wt[:], in_=wtile_d[:])

            # piece sums of y and y*f (y*f on gpsimd; DVE is the critical engine)
            yf = big.tile([PT, FT], BF16, tag="yf")
            nc.gpsimd.tensor_tensor(out=yf[:], in0=yt[:], in1=ft[:], op=MULT)
            sy = small.tile([PT, NP], BF16, tag="sy")
            syf = small.tile([PT, NP], BF16, tag="syf")
            with nc.allow_low_precision("piece sums are <=52 adds; bf16 out keeps DVE 2x mode"):
                nc.vector.tensor_reduce(
                    out=sy[:], in_=yt[:].rearrange("p (n w) -> p n w", w=WB), axis=AX, op=ADD
                )
                nc.vector.tensor_reduce(
                    out=syf[:], in_=yf[:].rearrange("p (n w) -> p n w", w=WB), axis=AX, op=ADD
                )

            # e^f once; softplus(f+c_k) = ln(e^{c_k} * e^f + 1)
            ef = big.tile([PT, FT], BF16, tag="ef")
            nc.scalar.activation(out=ef[:], in_=ft[:], func=ACT.Exp)

            # per-node piece sums of softplus(f + c_k); T pieces
            T = small.tile([PT, NP * K], F32, tag="T")
            T3 = T[:].rearrange("p (n k) -> p n k", k=K)
            for k in range(K):
                sp = tmp.tile([PT, FT], BF16, tag="sp")
                nc.scalar.activation(
                    out=sp[:], in_=ef[:], func=ACT.Ln, bias=1.0, scale=es[:, k : k + 1]
                )
                spn = tmp.tile([PT, NP], BF16, tag="spn")
                with nc.allow_low_precision("piece sums are <=52 adds; bf16 out keeps DVE 2x mode"):
                    nc.vector.tensor_reduce(
                        out=spn[:], in_=sp[:].rearrange("p (n w) -> p n w", w=WB), axis=AX, op=ADD
                    )
                # T_k = spn - syf - c_k*sy   (per piece)
                t1 = tmp.tile([PT, NP], F32, tag="t1")
                nc.vector.scalar_tensor_tensor(
                    out=t1[:], in0=sy[:], scalar=cb[:, k : k + 1], in1=spn[:],
                    op0=MULT, op1=SUB,
                )
                # t1 = c_k*sy - spn  -> T_k = -(t1 + syf) ... avoid negation:
                # instead compute T_k = (sy*(-c_k) + spn) - syf with negated scalar.
                # simpler: T_k = spn - syf - c_k*sy done in two ops:
                nc.vector.tensor_tensor(out=T3[:, :, k], in0=t1[:], in1=syf[:], op=ADD)
            # NOTE: T currently holds  c_k*sy - spn + syf  = -(T_k). We work with
            # negT below: ksum = sum_k w_k * exp(negT - negTmax), loss_q = -negTmax - ln ksum.
            negT = T

            # piece combine: PC_j = negT_j + m_j*(negT_{j+1} + m_{j+1}*negT_{j+2})
            # (supports groups spanning up to 3 pieces; host asserts that)
            negT_s = small.tile([PT, (NP + 1) * K], F32, tag="negT_s")
            nc.vector.memset(negT_s[:, NP * K :], 0.0)
            nc.vector.tensor_copy(out=negT_s[:, : NP * K], in_=negT[:])
            pc1 = small.tile([PT, NP * K], F32, tag="pc1")
            t2 = tmp.tile([PT, NP * K], F32, tag="t2")
            nc.vector.tensor_tensor(out=t2[:], in0=negT_s[:, K : (NP + 1) * K], in1=m5[:], op=MULT)
            nc.vector.tensor_tensor(out=pc1[:], in0=negT[:], in1=t2[:], op=ADD)
            pc1_s = small.tile([PT, (NP + 1) * K], F32, tag="pc1_s")
            nc.vector.memset(pc1_s[:, NP * K :], 0.0)
            nc.vector.tensor_copy(out=pc1_s[:, : NP * K], in_=pc1[:])
            pc = small.tile([PT, NP * K], F32, tag="pc")
            t3 = tmp.tile([PT, NP * K], F32, tag="t3")
            nc.vector.tensor_tensor(out=t3[:], in0=pc1_s[:, K : (NP + 1) * K], in1=m5[:], op=MULT)
            nc.vector.tensor_tensor(out=pc[:], in0=negT[:], in1=t3[:], op=ADD)

            # stabilized LSE over k on combined pieces
            nmax = small.tile([PT, NP], F32, tag="nmax")
            nc.vector.tensor_reduce(
                out=nmax[:], in_=pc[:].rearrange("p (n k) -> p n k", k=K),
                axis=AX, op=mybir.AluOpType.max,
            )
            pc3 = pc[:].rearrange("p (n k) -> p n k", k=K)
            for k in range(K):
                nc.vector.tensor_tensor(out=pc3[:, :, k], in0=pc3[:, :, k], in1=nmax[:], op=SUB)
            ex = small.tile([PT, NP * K], F32, tag="ex")
            nc.scalar.activation(out=ex[:], in_=pc[:], func=ACT.Exp)
            wm = small.tile([PT, NP * K], F32, tag="wm")
            nc.vector.tensor_tensor(out=wm[:], in0=ex[:], in1=wt[:], op=MULT)
            ks = small.tile([PT, NP], F32, tag="ks")
            nc.vector.tensor_reduce(
                out=ks[:], in_=wm[:].rearrange("p (n k) -> p n k", k=K), axis=AX, op=ADD
            )
            lk = small.tile([PT, NP], F32, tag="lk")
            nc.scalar.activation(out=lk[:], in_=ks[:], func=ACT.Ln)
            # loss_q = -log ksum_full = -(nmax + lk)  => accumulate (nmax+lk), negate at end
            dq = small.tile([PT, NP], F32, tag="dq")
            nc.vector.tensor_tensor(out=dq[:], in0=nmax[:], in1=lk[:], op=ADD)
            dqz = small.tile([PT, NP], F32, tag="dqz")
            nc.vector.tensor_tensor(out=dqz[:], in0=dq[:], in1=zt[:], op=MULT)
            if dbg:
                nc.sync.dma_start(out=dbg_t["dbg_T"][:], in_=negT[:])
                nc.sync.dma_start(out=dbg_t["dbg_P"][:], in_=pc[:])
                nc.sync.dma_start(out=dbg_t["dbg_dq"][:], in_=dqz[:])
            rs = small.tile([PT, 1], F32, tag="rs")
            nc.vector.tensor_reduce(out=rs[:], in_=dqz[:], axis=AX, op=ADD)
            negones = small.tile([PT, 1], F32, tag="negones")
            nc.vector.memset(negones[:], -1.0)
            tot_p = psum.tile([1, 1], F32)
            nc.tensor.matmul(out=tot_p[:], lhsT=rs[:], rhs=negones[:], start=True, stop=True)
            tot = small.tile([1, 1], F32, tag="tot")
            nc.vector.tensor_copy(out=tot[:], in_=tot_p[:])
            nc.sync.dma_start(out=loss_d[:], in_=tot[:])

    nc.compile()
    return nc


_NC_CACHE = {}


def get_nc(dbg=False):
    if dbg not in _NC_CACHE:
        _NC_CACHE[dbg] = build_nc(dbg)
    return _NC_CACHE[dbg]


def host_prep(y_true, y_pred, Z_idx, sig2b):
    """Sort by group; pack groups into fixed-width pieces across 1024 partitions."""
    y = np.asarray(y_true, dtype=np.float32).reshape(-1)
    f = np.asarray(y_pred, dtype=np.float32).reshape(-1)
    idx = np.asarray(Z_idx).astype(np.int32)
    n = y.shape[0]
    assert n == N

    perm = np.argsort(idx, kind="stable")
    sb = idx[perm]
    ys = y[perm]
    fs = f[perm]

    s = np.bincount(sb, minlength=Q).astype(np.int64)          # group sizes
    bin_start = np.concatenate(([0], np.cumsum(s)[:-1]))
    pcs = (s + WB - 1) // WB                                   # pieces per group
    piece_off = np.concatenate(([0], np.cumsum(pcs)[:-1]))
    total_pieces = int(pcs.sum())
    npt = -(-total_pieces // NPART)                            # target pieces/partition
    assert npt + int(pcs.max()) - 1 <= NP, (npt, int(pcs.max()))
    assert int(pcs.max()) <= 3, int(pcs.max())                 # combine depth

    nz = s > 0
    pid = np.zeros(Q, np.int64)
    pid[nz] = piece_off[nz] // npt                             # partition of each group
    assert pid.max() < NPART

    # local piece base per group: piece_off - first piece_off in its partition
    first_bin = np.searchsorted(pid[nz], np.arange(NPART), side="left")
    po_nz = piece_off[nz]
    part_first = np.zeros(NPART, np.int64)
    valid = first_bin < po_nz.shape[0]
    part_first[valid] = po_nz[np.minimum(first_bin, po_nz.shape[0] - 1)][valid]
    lpi = np.zeros(Q, np.int64)
    lpi[nz] = piece_off[nz] - part_first[pid[nz]]
    assert (lpi[nz] + pcs[nz]).max() <= NP

    # per-element slot
    b = sb.astype(np.int64)
    r = np.arange(n, dtype=np.int64) - bin_start[b]            # rank within group
    slot = pid[b] * FT + (lpi[b] + r // WB) * WB + (r % WB)

    Y = np.zeros(NPART * FT, np.float32)
    F = np.full(NPART * FT, FPAD, np.float32)
    Y[slot] = ys
    F[slot] = fs

    # masks
    mflat = np.zeros(NPART * NP, np.float32)
    for extra in (1, 2):
        sel = pcs > extra
        mflat[(pid[sel] * NP + lpi[sel] + (extra - 1)).astype(np.int64)] = 1.0
    zflat = np.zeros(NPART * NP, np.float32)
    zflat[(pid[nz] * NP + lpi[nz]).astype(np.int64)] = 1.0

    sig = float(np.asarray(sig2b).reshape(-1)[0])
    ck = (np.sqrt(2.0 * sig) * _XK).astype(np.float32)
    wk = (_WK / np.sqrt(np.pi)).astype(np.float32)
    cbias = np.tile(ck[None, :], (PT, 1)).astype(np.float32)
    escale = np.tile(np.exp(ck.astype(np.float64))[None, :], (PT, 1)).astype(np.float32)
    wtile = np.tile(wk[None, :], (PT, NP)).astype(np.float32)

    bf16 = mybir.dt.np(BF16)
    Y = Y.reshape(NPART, FT).astype(bf16)
    F = F.reshape(NPART, FT).astype(bf16)
    m5 = np.repeat(mflat.reshape(NPART, NP), K, axis=1)        # [NPART, NP*K]
    z2 = zflat.reshape(NPART, NP)

    in_maps = []
    for c in range(NCORES):
        sl = slice(c * PT, (c + 1) * PT)
        in_maps.append(
            {
                "ys": Y[sl],
                "fs": F[sl],
                "m5": m5[sl],
                "z": z2[sl],
                "cbias": cbias,
                "escale": escale,
                "wtile": wtile,
            }
        )
    return in_maps


def finish(results):
    total = sum(float(results[c]["loss"][0, 0]) for c in range(NCORES))
    return np.float32(total)


def kernel(y_true, y_pred, Z_idx, sig2b):
    nc = get_nc()
    in_maps = host_prep(y_true, y_pred, Z_idx, sig2b)
    res = run_bass_kernel_spmd(nc, in_maps, list(range(NCORES)))
    return finish(res.results)


# revision 16
# speedup vs baseline: 2.4542x; 1.0494x over previous
"""Trainium2 Bass kernel for nn_NLL_87333864997268 (GLMM logistic NLL with
Gauss-Hermite quadrature over a random intercept).

Math
----
With y in {0,1}, f the logit, c_k = sqrt(2*sig2b)*x_k (GH nodes):

    T[k,q] = sum_{i in group q} [ softplus(f_i + c_k) - y_i*(f_i + c_k) ]
           = SP_k[q] - YF[q] - c_k*SY[q]        (all three are segment sums)
    loss = -sum_q log( sum_k w_k/sqrt(pi) * exp(-T[k,q]) )

Strategy
--------
Host: stable sort by group id; pad each group to fixed-width W=52 "pieces"
(ceil(size/52) pieces per group, ~+37% slots); pack pieces into 1024
partition rows (8 cores x 128 partitions), never splitting a group across
partitions. All segment sums then become dense fixed-stride reductions on
device; host supplies two {0,1} masks: m (piece j continues into j+1) and
z (piece j is the first piece of its group).

Device (per core): 3 dense reduces for y and y*f, then per quadrature node
softplus (exp+ln, no Softplus table in this build) and a dense piece-sum
reduce; combine pieces via the m-mask (twice, supporting groups up to 3
pieces); stabilized log-sum-exp over the 5 nodes; z-masked total -> one
scalar per core. Host sums the 8 partials. Pad slots use f=-1e4, y=0 so
softplus(f+c)=0 exactly and they contribute nothing.
"""

import numpy as np

import concourse.bacc as bacc
import concourse.bass as bass
import concourse.mybir as mybir
import concourse.tile as tile
from concourse.bass_utils import run_bass_kernel_spmd

# problem constants (hardcoded per spec)
N = 4_194_304
Q = 100_000
NCORES = 8
K = 5            # Gauss-Hermite nodes
PT = 128         # partitions per core
NPART = NCORES * PT

WB = 52          # piece width (slots per piece)
NP = 108         # pieces per partition (static capacity)
FT = NP * WB     # slots per partition = 5616
FPAD = -10000.0  # pad logit: softplus(fpad+c)=0, y=0

_XK, _WK = np.polynomial.hermite.hermgauss(K)

F32 = mybir.dt.float32
BF16 = mybir.dt.bfloat16


def build_nc(dbg=False):
    """Build + compile the single-core SPMD Bass program."""
    nc = bacc.Bacc("TRN2", target_bir_lowering=False, debug=False)

    ys_d = nc.dram_tensor("ys", [PT, FT], BF16, kind="ExternalInput")
    fs_d = nc.dram_tensor("fs", [PT, FT], BF16, kind="ExternalInput")
    m5_d = nc.dram_tensor("m5", [PT, NP * K], F32, kind="ExternalInput")
    z_d = nc.dram_tensor("z", [PT, NP], F32, kind="ExternalInput")
    cbias_d = nc.dram_tensor("cbias", [PT, K], F32, kind="ExternalInput")
    escale_d = nc.dram_tensor("escale", [PT, K], F32, kind="ExternalInput")
    wtile_d = nc.dram_tensor("wtile", [PT, NP * K], F32, kind="ExternalInput")
    loss_d = nc.dram_tensor("loss", [1, 1], F32, kind="ExternalOutput")
    dbg_t = {}
    if dbg:
        dbg_t["dbg_T"] = nc.dram_tensor("dbg_T", [PT, NP * K], F32, kind="ExternalOutput")
        dbg_t["dbg_P"] = nc.dram_tensor("dbg_P", [PT, NP * K], F32, kind="ExternalOutput")
        dbg_t["dbg_dq"] = nc.dram_tensor("dbg_dq", [PT, NP], F32, kind="ExternalOutput")

    ADD = mybir.AluOpType.add
    SUB = mybir.AluOpType.subtract
    MULT = mybir.AluOpType.mult
    MIN = mybir.AluOpType.min
    ACT = mybir.ActivationFunctionType
    AX = mybir.AxisListType.X

    with tile.TileContext(nc) as tc:
        with (
            tc.tile_pool(name="big", bufs=1) as big,
            tc.tile_pool(name="tmp", bufs=2) as tmp,
            tc.tile_pool(name="small", bufs=1) as small,
            tc.tile_pool(name="psum", bufs=1, space="PSUM") as psum,
        ):
            yt = big.tile([PT, FT], BF16, tag="yt")
            ft = big.tile([PT, FT], BF16, tag="ft")
            nc.sync.dma_start(out=yt[:], in_=ys_d[:])
            nc.sync.dma_start(out=ft[:], in_=fs_d[:])

            cb = small.tile([PT, K], F32, tag="cb")
            es = small.tile([PT, K], F32, tag="es")
            m5 = small.tile([PT, NP * K], F32, tag="m5")
            zt = small.tile([PT, NP], F32, tag="zt")
            wt = small.tile([PT, NP * K], F32, tag="wt")
            nc.sync.dma_start(out=cb[:], in_=cbias_d[:])
            nc.sync.dma_start(out=es[:], in_=escale_d[:])
            nc.sync.dma_start(out=m5[:], in_=m5_d[:])
            nc.sync.dma_start(out=zt[:], in_=z_d[:])
            nc.sync.dma_start(out=wt[:], in_=wtile_d[:])

            # ---- big elementwise + piece-sum stage, in CH chunks for pipelining.
            # Piece sums via pairwise-halving tree (bf16 tensor_tensor hits the
            # DVE 2x packed mode; tensor_reduce never does) + final 13-reduce.
            CH = 2
            FC = FT // CH          # slots per chunk (2808)
            NC_ = NP // CH         # pieces per chunk (54)
            assert FC * CH == FT and NC_ * CH == NP

            sy = small.tile([PT, NP], BF16, tag="sy")
            syf = small.tile([PT, NP], BF16, tag="syf")
            T = small.tile([PT, NP * K], F32, tag="T")
            T3 = T[:].rearrange("p (n k) -> p n k", k=K)

            def tree_sum(src_ap, out_ap, npieces, l1_engine=nc.vector):
                """src [PT, npieces*WB] bf16 -> out [PT, npieces] bf16 piece sums."""
                s3 = src_ap.rearrange("p (n w) -> p n w", w=WB)
                h1 = tmp.tile([PT, npieces * 26], BF16, tag="h1")
                h13 = tmp.tile([PT, npieces * 13], BF16, tag="h13")
                h1v = h1[:].rearrange("p (n w) -> p n w", w=26)
                h13v = h13[:].rearrange("p (n w) -> p n w", w=13)
                l1_engine.tensor_tensor(out=h1v, in0=s3[:, :, 0:26], in1=s3[:, :, 26:52], op=ADD)
                nc.vector.tensor_tensor(out=h13v, in0=h1v[:, :, 0:13], in1=h1v[:, :, 13:26], op=ADD)
                nc.vector.tensor_reduce(out=out_ap, in_=h13v, axis=AX, op=ADD)

            with nc.allow_low_precision("piece sums are <=52 adds; bf16 keeps DVE 2x mode"):
                for c in range(CH):
                    fsl = slice(c * FC, (c + 1) * FC)
                    nsl = slice(c * NC_, (c + 1) * NC_)
                    # y and y*f piece sums (first tree level on gpsimd)
                    yf = tmp.tile([PT, FC], BF16, tag="yf")
                    nc.gpsimd.tensor_tensor(out=yf[:], in0=yt[:, fsl], in1=ft[:, fsl], op=MULT)
                    tree_sum(yt[:, fsl], sy[:, nsl], NC_, l1_engine=nc.gpsimd)
                    tree_sum(yf[:], syf[:, nsl], NC_, l1_engine=nc.gpsimd)

                    # e^f once per chunk; softplus(f+c_k) = ln(e^{c_k} * e^f + 1)
                    ef = tmp.tile([PT, FC], BF16, tag="ef")
                    nc.scalar.activation(out=ef[:], in_=ft[:, fsl], func=ACT.Exp)
                    for k in range(K):
                        sp = tmp.tile([PT, FC], BF16, tag="sp")
                        nc.scalar.activation(
                            out=sp[:], in_=ef[:], func=ACT.Ln, bias=1.0,
                            scale=es[:, k : k + 1],
                        )
                        spn = tmp.tile([PT, NC_], BF16, tag="spn")
                        tree_sum(sp[:], spn[:], NC_)
                        # T_k = spn - syf - c_k*sy  stored negated: c_k*sy - spn + syf
                        t1 = tmp.tile([PT, NC_], F32, tag="t1")
                        nc.vector.scalar_tensor_tensor(
                            out=t1[:], in0=sy[:, nsl], scalar=cb[:, k : k + 1],
                            in1=spn[:], op0=MULT, op1=SUB,
                        )
                        nc.vector.tensor_tensor(
                            out=T3[:, nsl, k], in0=t1[:], in1=syf[:, nsl], op=ADD
                        )
            # NOTE: T currently holds  c_k*sy - spn + syf  = -(T_k). We work with
            # negT below: ksum = sum_k w_k * exp(negT - negTmax), loss_q = -negTmax - ln ksum.
            negT = T

            # piece combine: PC_j = negT_j + m_j*(negT_{j+1} + m_{j+1}*negT_{j+2})
            # (supports groups spanning up to 3 pieces; host asserts that)
            negT_s = small.tile([PT, (NP + 1) * K], F32, tag="negT_s")
            nc.vector.memset(negT_s[:, NP * K :], 0.0)
            nc.vector.tensor_copy(out=negT_s[:, : NP * K], in_=negT[:])
            pc1 = small.tile([PT, NP * K], F32, tag="pc1")
            t2 = tmp.tile([PT, NP * K], F32, tag="t2")
            nc.vector.tensor_tensor(out=t2[:], in0=negT_s[:, K : (NP + 1) * K], in1=m5[:], op=MULT)
            nc.vector.tensor_tensor(out=pc1[:], in0=negT[:], in1=t2[:], op=ADD)
            pc1_s = small.tile([PT, (NP + 1) * K], F32, tag="pc1_s")
            nc.vector.memset(pc1_s[:, NP * K :], 0.0)
            nc.vector.tensor_copy(out=pc1_s[:, : NP * K], in_=pc1[:])
            pc = small.tile([PT, NP * K], F32, tag="pc")
            t3 = tmp.tile([PT, NP * K], F32, tag="t3")
            nc.vector.tensor_tensor(out=t3[:], in0=pc1_s[:, K : (NP + 1) * K], in1=m5[:], op=MULT)
            nc.vector.tensor_tensor(out=pc[:], in0=negT[:], in1=t3[:], op=ADD)

            # stabilized LSE over k on combined pieces
            nmax = small.tile([PT, NP], F32, tag="nmax")
            nc.vector.tensor_reduce(
                out=nmax[:], in_=pc[:].rearrange("p (n k) -> p n k", k=K),
                axis=AX, op=mybir.AluOpType.max,
            )
            pc3 = pc[:].rearrange("p (n k) -> p n k", k=K)
            for k in range(K):
                nc.vector.tensor_tensor(out=pc3[:, :, k], in0=pc3[:, :, k], in1=nmax[:], op=SUB)
            ex = small.tile([PT, NP * K], F32, tag="ex")
            nc.scalar.activation(out=ex[:], in_=pc[:], func=ACT.Exp)
            wm = small.tile([PT, NP * K], F32, tag="wm")
            nc.vector.tensor_tensor(out=wm[:], in0=ex[:], in1=wt[:], op=MULT)
            ks = small.tile([PT, NP], F32, tag="ks")
            nc.vector.tensor_reduce(
                out=ks[:], in_=wm[:].rearrange("p (n k) -> p n k", k=K), axis=AX, op=ADD
            )
            lk = small.tile([PT, NP], F32, tag="lk")
            nc.scalar.activation(out=lk[:], in_=ks[:], func=ACT.Ln)
            # loss_q = -log ksum_full = -(nmax + lk)  => accumulate (nmax+lk), negate at end
            dq = small.tile([PT, NP], F32, tag="dq")
            nc.vector.tensor_tensor(out=dq[:], in0=nmax[:], in1=lk[:], op=ADD)
            dqz = small.tile([PT, NP], F32, tag="dqz")
            nc.vector.tensor_tensor(out=dqz[:], in0=dq[:], in1=zt[:], op=MULT)
            if dbg:
                nc.sync.dma_start(out=dbg_t["dbg_T"][:], in_=negT[:])
                nc.sync.dma_start(out=dbg_t["dbg_P"][:], in_=pc[:])
                nc.sync.dma_start(out=dbg_t["dbg_dq"][:], in_=dqz[:])
            rs = small.tile([PT, 1], F32, tag="rs")
            nc.vector.tensor_reduce(out=rs[:], in_=dqz[:], axis=AX, op=ADD)
            negones = small.tile([PT, 1], F32, tag="negones")
            nc.vector.memset(negones[:], -1.0)
            tot_p = psum.tile([1, 1], F32)
            nc.tensor.matmul(out=tot_p[:], lhsT=rs[:], rhs=negones[:], start=True, stop=True)
            tot = small.tile([1, 1], F32, tag="tot")
            nc.vector.tensor_copy(out=tot[:], in_=tot_p[:])
            nc.sync.dma_start(out=loss_d[:], in_=tot[:])

    nc.compile()
    return nc


_NC_CACHE = {}


def get_nc(dbg=False):
    if dbg not in _NC_CACHE:
        _NC_CACHE[dbg] = build_nc(dbg)
    return _NC_CACHE[dbg]


def host_prep(y_true, y_pred, Z_idx, sig2b):
    """Sort by group; pack groups into fixed-width pieces across 1024 partitions."""
    y = np.asarray(y_true, dtype=np.float32).reshape(-1)
    f = np.asarray(y_pred, dtype=np.float32).reshape(-1)
    idx = np.asarray(Z_idx).astype(np.int32)
    n = y.shape[0]
    assert n == N

    perm = np.argsort(idx, kind="stable")
    sb = idx[perm]
    ys = y[perm]
    fs = f[perm]

    s = np.bincount(sb, minlength=Q).astype(np.int64)          # group sizes
    bin_start = np.concatenate(([0], np.cumsum(s)[:-1]))
    pcs = (s + WB - 1) // WB                                   # pieces per group
    piece_off = np.concatenate(([0], np.cumsum(pcs)[:-1]))
    total_pieces = int(pcs.sum())
    npt = -(-total_pieces // NPART)                            # target pieces/partition
    assert npt + int(pcs.max()) - 1 <= NP, (npt, int(pcs.max()))
    assert int(pcs.max()) <= 3, int(pcs.max())                 # combine depth

    nz = s > 0
    pid = np.zeros(Q, np.int64)
    pid[nz] = piece_off[nz] // npt                             # partition of each group
    assert pid.max() < NPART

    # local piece base per group: piece_off - first piece_off in its partition
    first_bin = np.searchsorted(pid[nz], np.arange(NPART), side="left")
    po_nz = piece_off[nz]
    part_first = np.zeros(NPART, np.int64)
    valid = first_bin < po_nz.shape[0]
    part_first[valid] = po_nz[np.minimum(first_bin, po_nz.shape[0] - 1)][valid]
    lpi = np.zeros(Q, np.int64)
    lpi[nz] = piece_off[nz] - part_first[pid[nz]]
    assert (lpi[nz] + pcs[nz]).max() <= NP

    # per-element slot
    b = sb.astype(np.int64)
    r = np.arange(n, dtype=np.int64) - bin_start[b]            # rank within group
    slot = pid[b] * FT + (lpi[b] + r // WB) * WB + (r % WB)

    Y = np.zeros(NPART * FT, np.float32)
    F = np.full(NPART * FT, FPAD, np.float32)
    Y[slot] = ys
    F[slot] = fs

    # masks
    mflat = np.zeros(NPART * NP, np.float32)
    for extra in (1, 2):
        sel = pcs > extra
        mflat[(pid[sel] * NP + lpi[sel] + (extra - 1)).astype(np.int64)] = 1.0
    zflat = np.zeros(NPART * NP, np.float32)
    zflat[(pid[nz] * NP + lpi[nz]).astype(np.int64)] = 1.0

    sig = float(np.asarray(sig2b).reshape(-1)[0])
    ck = (np.sqrt(2.0 * sig) * _XK).astype(np.float32)
    wk = (_WK / np.sqrt(np.pi)).astype(np.float32)
    cbias = np.tile(ck[None, :], (PT, 1)).astype(np.float32)
    escale = np.tile(np.exp(ck.astype(np.float64))[None, :], (PT, 1)).astype(np.float32)
    wtile = np.tile(wk[None, :], (PT, NP)).astype(np.float32)

    bf16 = mybir.dt.np(BF16)
    Y = Y.reshape(NPART, FT).astype(bf16)
    F = F.reshape(NPART, FT).astype(bf16)
    m5 = np.repeat(mflat.reshape(NPART, NP), K, axis=1)        # [NPART, NP*K]
    z2 = zflat.reshape(NPART, NP)

    in_maps = []
    for c in range(NCORES):
        sl = slice(c * PT, (c + 1) * PT)
        in_maps.append(
            {
                "ys": Y[sl],
                "fs": F[sl],
                "m5": m5[sl],
                "z": z2[sl],
                "cbias": cbias,
                "escale": escale,
                "wtile": wtile,
            }
        )
    return in_maps


def finish(results):
    total = sum(float(results[c]["loss"][0, 0]) for c in range(NCORES))
    return np.float32(total)


def kernel(y_true, y_pred, Z_idx, sig2b):
    nc = get_nc()
    in_maps = host_prep(y_true, y_pred, Z_idx, sig2b)
    res = run_bass_kernel_spmd(nc, in_maps, list(range(NCORES)))
    return finish(res.results)
